# revision 81
# baseline (speedup 1.0000x reference)
"""Trainium2 Bass kernel for a 6-layer post-LN transformer encoder.

Problem: B=8, S=1024, H=1024, NH=16 heads (HD=64), PF=4096, L=6, V=32000.
Sharding: pure data-parallel over batch -- each of the 8 NeuronCores runs one
batch element end-to-end (no collectives).

Device-side layout: activations are kept TRANSPOSED, xT[h, s], stored in SBUF
as [128 partitions, 8 h-tiles, 1024 s].  With weights in natural [h_in, h_out]
layout as the stationary matmul operand, every projection's output comes out
transposed as well, so the entire layer chains matmuls with zero on-device
transposes:
  qT/kT  = W.T @ xT            (lhsT = W[h,j],  rhs = xT)         -> [j, s]
  v      = xT.T @ Wv           (lhsT = xT tile, rhs = Wv[h,j])    -> [s, j]
  scoresT= kT_head.T @ qT_head (K = 64 head dims)                 -> [k, q]
  expT   = exp(scoresT / 8)    (no max-subtraction needed: |scores|<~3)
  oT,Z   = [v_head|1].T @ expT (ones column gives softmax denom Z) -> [d+1, q]
  attn   = Wo.T @ oT ;  FFN the same way.
LayerNorm is over the partition dim; stats come from ones-vector matmuls and
rstd from a DVE Newton rsqrt (ScalarE never leaves the exp table set).  The
LN affine is DEFERRED into the consumers' PSUM copy-outs (x2 @ W =
r*(t @ Wg) - (m*r)*colsum(Wg) with g/c folded into W/b on the host), so
TensorE runs the next phase's matmuls on the pre-LN tensor and never waits
for the LN chain.  Matmuls run in bf16 with fp32 PSUM accumulation; the
residual stream is fp32.
"""

import sys

import numpy as np

for _p in ("/opt/pypackages", "/opt/trn_rl_repo"):
    if _p not in sys.path:
        sys.path.insert(0, _p)

import ml_dtypes  # noqa: E402

import concourse.bass as bass  # noqa: E402
import concourse.bacc as bacc  # noqa: E402
import concourse.mybir as mybir  # noqa: E402
import concourse.tile as tile  # noqa: E402
from concourse.bass_utils import run_bass_kernel_spmd  # noqa: E402
from concourse.masks import make_identity  # noqa: E402

B, S, H, NH, PF, L, V = 8, 1024, 1024, 16, 4096, 6, 32000
HD = H // NH  # 64
P = 128
HT = H // P  # 8 h-tiles
PT = PF // P  # 32 pf-tiles
NSC = 2
SCW = S // NSC  # 512
NPAIR = NH // 2  # 8 head pairs
EPS = 1e-5

f32 = mybir.dt.float32
bf16 = mybir.dt.bfloat16
i32 = mybir.dt.int32
OP = mybir.AluOpType
AF = mybir.ActivationFunctionType


def _bcast_row(nc, out_ap, row_ap, rows):
    """Replicate a [1, N] SBUF row across `rows` partitions of out_ap."""
    nc.gpsimd.partition_broadcast(out_ap, row_ap)


def build_nc():
    nc = bacc.Bacc("TRN2")

    src_t = nc.dram_tensor("src", [S], i32, kind="ExternalInput")
    emb_t = nc.dram_tensor("emb", [V, H], f32, kind="ExternalInput")
    pos_t = nc.dram_tensor("post", [P, HT, S], bf16, kind="ExternalInput")
    wq_t = nc.dram_tensor("wq", [L, HT, P, HT, P], bf16, kind="ExternalInput")
    wk_t = nc.dram_tensor("wk", [L, HT, P, HT, P], bf16, kind="ExternalInput")
    wv_t = nc.dram_tensor("wv", [L, P, HT, H], bf16, kind="ExternalInput")
    # negated column sums of (g2-folded) Wv for the deferred-LN V projection
    wvs_t = nc.dram_tensor("wvs", [L, H], bf16, kind="ExternalInput")
    wo_t = nc.dram_tensor("wo", [L, HT, P, HT, P], bf16, kind="ExternalInput")
    w1_t = nc.dram_tensor("w1", [L, PT, P, HT, P], bf16, kind="ExternalInput")
    w2_t = nc.dram_tensor("w2", [L, HT, P, PT, P], bf16, kind="ExternalInput")
    bq_t = nc.dram_tensor("bq", [L, P, HT], f32, kind="ExternalInput")
    bk_t = nc.dram_tensor("bk", [L, P, HT], f32, kind="ExternalInput")
    bo_t = nc.dram_tensor("bo", [L, P, HT], f32, kind="ExternalInput")
    b1_t = nc.dram_tensor("b1", [L, P, PT], f32, kind="ExternalInput")
    w1s_t = nc.dram_tensor("w1s", [L, P, PT], f32, kind="ExternalInput")
    wqs_t = nc.dram_tensor("wqs", [L, P, HT], f32, kind="ExternalInput")
    wks_t = nc.dram_tensor("wks", [L, P, HT], f32, kind="ExternalInput")
    b2_t = nc.dram_tensor("b2", [L, P, HT], f32, kind="ExternalInput")
    g1_t = nc.dram_tensor("g1", [L, P, HT], f32, kind="ExternalInput")
    c1_t = nc.dram_tensor("c1", [L, P, HT], f32, kind="ExternalInput")
    g2_t = nc.dram_tensor("g2", [L, P, HT], f32, kind="ExternalInput")
    c2_t = nc.dram_tensor("c2", [L, P, HT], f32, kind="ExternalInput")
    # bf16 output (host upcasts): halves the store DMA and lets the final
    # tensor_scalar cast in the same pass
    out_t = nc.dram_tensor("outT", [P, HT, S], bf16, kind="ExternalOutput")

    from contextlib import ExitStack

    with tile.TileContext(nc) as tc:
        with ExitStack() as ctx:
            constp = ctx.enter_context(tc.tile_pool(name="const", bufs=1))
            residp = ctx.enter_context(tc.tile_pool(name="resid", bufs=1))
            bigp = ctx.enter_context(tc.tile_pool(name="big", bufs=1))
            ktp = ctx.enter_context(tc.tile_pool(name="ktx2", bufs=1))
            tb2p = ctx.enter_context(tc.tile_pool(name="tb2", bufs=1))
            csp = ctx.enter_context(tc.tile_pool(name="cs", bufs=2))
            vaugp = ctx.enter_context(tc.tile_pool(name="vaug", bufs=1))
            xbfop = ctx.enter_context(tc.tile_pool(name="xbfo", bufs=1))
            expp = ctx.enter_context(tc.tile_pool(name="expp", bufs=1))
            wp8 = ctx.enter_context(tc.tile_pool(name="wp8", bufs=4))
            wop = ctx.enter_context(tc.tile_pool(name="wop", bufs=3))
            wpv = ctx.enter_context(tc.tile_pool(name="wpv", bufs=2))
            wp2 = ctx.enter_context(tc.tile_pool(name="wp2", bufs=2))
            rowp = ctx.enter_context(tc.tile_pool(name="rowp", bufs=1))
            rowp2 = ctx.enter_context(tc.tile_pool(name="rowp2", bufs=1))
            biasp = ctx.enter_context(tc.tile_pool(name="biasp", bufs=1))
            biasp2 = ctx.enter_context(tc.tile_pool(name="biasp2", bufs=2))
            psp = ctx.enter_context(tc.tile_pool(name="ps", bufs=2, space="PSUM"))
            pssc = ctx.enter_context(tc.tile_pool(name="ps_sc", bufs=2, space="PSUM"))
            psst = ctx.enter_context(tc.tile_pool(name="ps_st", bufs=2, space="PSUM"))
            ident = constp.tile([P, P], f32)
            make_identity(nc, ident[:])
            ones_bf = constp.tile([P, 1], bf16)
            nc.vector.memset(ones_bf[:], 1.0)

            xT = residp.tile([P, HT, S], f32)  # residual stream, updated in place

            # dependency-free warm-up matmuls: run during the first embedding
            # gather's ~15us DMA wait so the PE clock-gate (HAM) is already at
            # full rate when the real transposes arrive
            warm = psp.tile([P, 512], f32, tag="mm", name="warm")
            for _ in range(12):
                nc.tensor.transpose(warm[:, :P], ident[:], ident[:])

            def _emb_tile(g, st, posT, x_bf):
                # 4 transposes batched per PSUM tile; evac on ACT (idle here)
                # so the DVE only does the cheap pos add -- keeps the
                # embedding from being DVE- or evac-bound
                ssl = slice(st * P, (st + 1) * P)
                for hg in range(2):
                    pst = psp.tile([P, 512], f32, tag="mm")
                    for hi in range(4):
                        ht = hg * 4 + hi
                        nc.tensor.transpose(pst[:, hi * P:(hi + 1) * P],
                                            g[:, ht * P:(ht + 1) * P], ident[:])
                    hsl = slice(hg * 4, (hg + 1) * 4)
                    if (st + hg) % 2 == 0:
                        nc.scalar.activation(
                            out=xT[:, hsl, ssl],
                            in_=pst[:].rearrange("p (a b) -> p a b", b=P),
                            func=AF.Copy, bias=0.0, scale=float(np.sqrt(H)))
                        nc.vector.tensor_tensor(
                            out=xT[:, hsl, ssl], in0=xT[:, hsl, ssl],
                            in1=posT[:, hsl, ssl], op=OP.add)
                    else:
                        # alternate the PSUM evac between ACT and DVE so
                        # neither engine gates the transpose pipeline
                        nc.vector.scalar_tensor_tensor(
                            out=xT[:, hsl, ssl],
                            in0=pst[:].rearrange("p (a b) -> p a b", b=P),
                            scalar=float(np.sqrt(H)), in1=posT[:, hsl, ssl],
                            op0=OP.mult, op1=OP.add)

            # ---------------- embedding ----------------
            # indirect gathers first (slow per-row descriptors); pos DMA after
            # so it doesn't delay the first transpose's input
            posT = ktp.tile([P, HT, S], bf16, tag="ktx2")
            x_bf = xbfop.tile([P, HT, S], bf16, tag="xbfo")
            g_tiles = []
            for st in range(HT):
                idx = biasp.tile([P, 1], i32, tag="idx", bufs=4)
                nc.sync.dma_start(out=idx[:], in_=src_t[st * P:(st + 1) * P, None])
                if st % 2 == 0:
                    g = bigp.tile([P, H], f32, tag="big")
                else:
                    g = vaugp.tile([P, H], f32, tag="vaug")
                nc.gpsimd.indirect_dma_start(
                    out=g[:],
                    out_offset=None,
                    in_=emb_t[:, :],
                    in_offset=bass.IndirectOffsetOnAxis(ap=idx[:, :1], axis=0),
                )
                g_tiles.append(g)
                if st == 1:
                    nc.sync.dma_start(out=posT[:], in_=pos_t[:, :, :])
                if st < 2:
                    continue
                _emb_tile(g_tiles[st - 2], st - 2, posT, x_bf)
            _emb_tile(g_tiles[HT - 2], HT - 2, posT, x_bf)
            _emb_tile(g_tiles[HT - 1], HT - 1, posT, x_bf)
            for ht in range(HT):
                nc.vector.tensor_copy(out=x_bf[:, ht, :], in_=xT[:, ht, :])

            def ln_begin(sc, tb_tag, sq_tag):
                """Allocate the chunk's tb/sq tiles and the two stat-psum
                accumulators so stats can be fed incrementally per h-tile."""
                if tb_tag == "tb2":
                    tb = tb2p.tile([P, HT, SCW], bf16, tag=f"tb2_{sc}")
                elif tb_tag in ("expA", "expB"):
                    tb = expp.tile([P, HT, SCW], bf16, tag=tb_tag)
                else:
                    tb = ktp.tile([P, HT, SCW], bf16, tag=tb_tag)
                if sq_tag == "ktx2":
                    sq = ktp.tile([P, HT, SCW], bf16, tag="ktx2", name="sq")
                else:
                    sq = bigp.tile([P, HT, SCW], bf16, tag="big", name="sq")
                psm = psst.tile([1, SCW], f32, tag="st")
                pss = psst.tile([1, SCW], f32, tag="st")
                return (tb, sq, psm, pss)

            def ln_kt(state, sc, kt, defer_mms=False):
                """Fold h-tile kt of this chunk into the LN stats: bf16 copy,
                square (2x DVE mode), and one accumulating matmul per stat.
                Emitted one h-tile behind the residual producer so the PE
                never waits on the DVE copies."""
                tb, sq, psm, pss = state
                ssl = slice(sc * SCW, (sc + 1) * SCW)
                nc.vector.tensor_copy(out=tb[:, kt, :], in_=xT[:, kt, ssl])
                nc.vector.tensor_tensor(
                    out=sq[:, kt, :], in0=tb[:, kt, :], in1=tb[:, kt, :],
                    op=OP.mult)
                if defer_mms:
                    return
                nc.tensor.matmul(psm[:], ones_bf[:], tb[:, kt, :],
                                 start=(kt == 0), stop=(kt == HT - 1))
                nc.tensor.matmul(pss[:], ones_bf[:], sq[:, kt, :],
                                 start=(kt == 0), stop=(kt == HT - 1))

            def ln_end(state, sc, rbpair, mrbpair, cols=None):
                """Stat rows -> rstd/mean*rstd -> partition broadcasts."""
                tb, sq, psm, pss = state
                if True:
                    mrow = rowp.tile([1, SCW], f32, tag="mrow")
                    nc.vector.tensor_scalar(out=mrow[:], in0=psm[:], scalar1=1.0 / H,
                                            scalar2=None, op0=OP.mult)
                    m2 = rowp.tile([1, SCW], f32, tag="lntmp")
                    nc.vector.tensor_tensor(out=m2[:], in0=mrow[:], in1=mrow[:], op=OP.mult)
                    vrow = rowp.tile([1, SCW], f32, tag="vrow")
                    nc.vector.scalar_tensor_tensor(
                        out=vrow[:], in0=pss[:], scalar=1.0 / H, in1=m2[:],
                        op0=OP.mult, op1=OP.subtract)
                    nc.vector.tensor_scalar(out=vrow[:], in0=vrow[:], scalar1=EPS,
                                            scalar2=None, op0=OP.add)
                    # rstd = rsqrt(vrow): bit-hack seed + 2 Newton iterations
                    yrow = rowp.tile([1, SCW], f32, tag="yrow")
                    nc.vector.tensor_scalar(
                        out=yrow[:].bitcast(i32), in0=vrow[:].bitcast(i32),
                        scalar1=1, scalar2=None, op0=OP.logical_shift_right)
                    nc.vector.tensor_scalar(
                        out=yrow[:].bitcast(i32), in0=yrow[:].bitcast(i32),
                        scalar1=-1, scalar2=0x5F3759DF, op0=OP.mult, op1=OP.add)
                    srow = rowp.tile([1, SCW], f32, tag="lntmp")
                    for _ in range(1):
                        nc.vector.tensor_tensor(out=srow[:], in0=yrow[:], in1=yrow[:],
                                                op=OP.mult)
                        nc.vector.tensor_tensor(out=srow[:], in0=srow[:], in1=vrow[:],
                                                op=OP.mult)
                        nc.vector.tensor_scalar(out=srow[:], in0=srow[:], scalar1=-0.5,
                                                scalar2=1.5, op0=OP.mult, op1=OP.add)
                        nc.vector.tensor_tensor(out=yrow[:], in0=yrow[:], in1=srow[:],
                                                op=OP.mult)
                    if cols is not None and cols[0] is not None:
                        # fp32 m*r row (ACT scale APs must be FP32) feeding the
                        # per-partition column copies for the next layer's
                        # deferred-LN V projection
                        mrf = rowp.tile([1, SCW], f32, tag="mrf")
                        nc.vector.tensor_tensor(out=mrf[:], in0=mrow[:],
                                                in1=yrow[:], op=OP.mult)
                        mrrow = rowp.tile([1, SCW], bf16, tag="mrow2")
                        nc.vector.tensor_copy(out=mrrow[:], in_=mrf[:])
                        rcol_n, mrcol_n = cols
                        for st4 in range(4):
                            st = sc * 4 + st4
                            wsl = slice(st4 * P, (st4 + 1) * P)
                            nc.sync.dma_start(out=rcol_n[:, st:st + 1],
                                              in_=yrow[0:1, wsl])
                            nc.sync.dma_start(out=mrcol_n[:, st:st + 1],
                                              in_=mrf[0:1, wsl])
                    else:
                        mrrow = rowp.tile([1, SCW], bf16, tag="mrow2")
                        nc.vector.tensor_tensor(out=mrrow[:], in0=mrow[:],
                                                in1=yrow[:], op=OP.mult)
                    ybrow = rowp.tile([1, SCW], bf16, tag="rz")
                    nc.vector.tensor_copy(out=ybrow[:], in_=yrow[:])
                    _bcast_row(nc, rbpair[:, sc, :], ybrow[:], P)
                    _bcast_row(nc, mrbpair[:, sc, :], mrrow[:], P)
                    return tb

            def emit_ln_apply(handles, g_sb, c_sb, out_bf, out_sb=None, scs=None):
                """In-place fp32 LN apply: x2 = (x*r - m*r)*g + c.

                With out_sb set (final layer), the affine result is written to
                the bf16 out_sb tile and DMA'd out instead of updating xT
                (dead after the last layer); scs restricts to given chunks so
                chunk 0 overlaps chunk 1's FFN2 matmuls."""
                final = out_sb is not None
                for sc in (range(NSC) if scs is None else scs):
                    ssl = slice(sc * SCW, (sc + 1) * SCW)
                    _tb, rb, mrb = handles[sc]
                    # per-tile ops; odd tiles go to the (mostly idle) GpSimd
                    # engine -- on the final layer chunk 0 goes there entirely
                    # so the apply hides under chunk 1's FFN2 matmuls
                    def _eng(kt):
                        if final:
                            pool = (sc == 0) or kt >= 5
                        else:
                            pool = kt % 2 == 1
                        return nc.gpsimd if pool else nc.vector
                    for kt in range(HT):
                        _eng(kt).tensor_tensor(
                            out=xT[:, kt, ssl], in0=xT[:, kt, ssl],
                            in1=rb[:], op=OP.mult)
                        _eng(kt).tensor_tensor(
                            out=xT[:, kt, ssl], in0=xT[:, kt, ssl],
                            in1=mrb[:], op=OP.subtract)
                    # critical path first: bf16 LN output for consuming matmuls
                    for kt in range(HT):
                        if out_bf is not None:
                            nc.vector.tensor_scalar(
                                out=out_bf[:, kt, ssl], in0=xT[:, kt, ssl],
                                scalar1=g_sb[:, kt:kt + 1], scalar2=c_sb[:, kt:kt + 1],
                                op0=OP.mult, op1=OP.add)
                    for kt in range(HT):
                        _eng(kt).tensor_scalar(
                            out=(out_sb if final else xT)[:, kt, ssl],
                            in0=xT[:, kt, ssl],
                            scalar1=g_sb[:, kt:kt + 1], scalar2=c_sb[:, kt:kt + 1],
                            op0=OP.mult, op1=OP.add)
                        if final and kt == 3:
                            # first half streams out while the second half's
                            # affine is still running on the other engine
                            nc.sync.dma_start(out=out_t[:, 0:4, ssl],
                                              in_=out_sb[:, 0:4, ssl])
                    if final:
                        nc.sync.dma_start(out=out_t[:, 4:HT, ssl],
                                          in_=out_sb[:, 4:HT, ssl])

            # ---------------- layers ----------------
            prev_ln2 = None
            pg2_sb = pc2_sb = None
            rcol_p = mrcol_p = None
            deferred_mms = None
            wv_next = None
            for l in range(L):
                bq_sb = biasp.tile([P, HT], f32, tag="bq")
                nc.sync.dma_start(out=bq_sb[:], in_=bq_t[l])
                bk_sb = biasp.tile([P, HT], f32, tag="bk")
                nc.sync.dma_start(out=bk_sb[:], in_=bk_t[l])
                bo_sb = biasp.tile([P, HT], f32, tag="bo")
                nc.sync.dma_start(out=bo_sb[:], in_=bo_t[l])
                b1_sb = biasp.tile([P, PT], f32, tag="b1")
                nc.sync.dma_start(out=b1_sb[:], in_=b1_t[l])
                w1s_sb = biasp.tile([P, PT], f32, tag="w1s")
                nc.sync.dma_start(out=w1s_sb[:], in_=w1s_t[l])
                wqs_sb = biasp.tile([P, HT], f32, tag="wqs")
                nc.sync.dma_start(out=wqs_sb[:], in_=wqs_t[l])
                wks_sb = biasp.tile([P, HT], f32, tag="wks")
                nc.sync.dma_start(out=wks_sb[:], in_=wks_t[l])
                b2_sb = biasp.tile([P, HT], f32, tag="b2")
                nc.sync.dma_start(out=b2_sb[:], in_=b2_t[l])
                g1_sb = biasp.tile([P, HT], f32, tag="g1")
                nc.sync.dma_start(out=g1_sb[:], in_=g1_t[l])
                c1_sb = biasp.tile([P, HT], f32, tag="c1")
                nc.sync.dma_start(out=c1_sb[:], in_=c1_t[l])
                g2_sb = biasp2.tile([P, HT], f32, tag="g2")
                nc.sync.dma_start(out=g2_sb[:], in_=g2_t[l])
                c2_sb = biasp2.tile([P, HT], f32, tag="c2")
                nc.sync.dma_start(out=c2_sb[:], in_=c2_t[l])

                # ---- phase A+B: V, then a software pipeline interleaving the
                # Q/K projections with the attention inner loop so the exp
                # work (ScalarE-bound) spreads across the whole phase instead
                # of gating the PE in a bunched scores->exp->attnV chain.
                #
                # For l>=1 everything runs on the pre-LN tensor from the
                # previous layer's LN2 with the affine deferred into the PSUM
                # copy-outs (Q/K: folded scale rows; V: r/mr columns + Wv
                # colsum row), so nothing waits for the LN apply chain.
                qT = bigp.tile([P, HT, S], bf16, tag="big")
                kT = ktp.tile([P, HT, S], bf16, tag="ktx2")
                oT = xbfop.tile([P, HT, S], bf16, tag="xbfo")

                v_aug = vaugp.tile([P, HT, NH, HD + 1], bf16, tag="vaug")
                nc.vector.memset(v_aug[:, :, :, HD:HD + 1], 1.0)
                if prev_ln2 is not None:
                    wvs_row = rowp.tile([1, H], bf16, tag="wvsrow")
                    nc.sync.dma_start(out=wvs_row[:], in_=wvs_t[l, None, :])
                    wvs_b = csp.tile([P, H], bf16, tag="csb")
                    _bcast_row(nc, wvs_b[:, :], wvs_row[:], P)
                for jc in range(NSC):
                    if wv_next is not None:
                        # prefetched during the previous layer's phase C
                        wv_sb = wv_next[jc]
                    else:
                        wv_sb = wpv.tile([P, HT, SCW], bf16, tag="wv")
                        # ACT DGE queue: not stuck behind the W1/W2 stream on SP
                        nc.scalar.dma_start(out=wv_sb[:], in_=wv_t[l, :, :, jc * SCW:(jc + 1) * SCW])
                    for st in range(HT):
                        pv = psp.tile([P, 512], f32, tag="mm")
                        sc_of = st // 4
                        for ko in range(HT):
                            lhs = (x_bf[:, ko, st * P:(st + 1) * P]
                                   if prev_ln2 is None else
                                   prev_ln2[sc_of][0][:, ko, (st % 4) * P:(st % 4 + 1) * P])
                            nc.tensor.matmul(pv[:], lhs, wv_sb[:, ko, :],
                                             start=(ko == 0), stop=(ko == HT - 1))
                        vsl = v_aug[:, st, jc * 8:(jc + 1) * 8, 0:HD]
                        if prev_ln2 is None:
                            nc.scalar.activation(
                                out=vsl, in_=pv[:].rearrange("p (g d) -> p g d", d=HD),
                                func=AF.Copy, bias=0.0, scale=1.0)
                        else:
                            # v = r[s]*(t @ Wv') - mr[s]*colsum(Wv'): scale on
                            # ACT (per-partition r column), rank-1 fixup on the
                            # idle GpSimd so the DVE stays clear for Q/K evacs
                            nc.scalar.activation(
                                out=vsl, in_=pv[:].rearrange("p (g d) -> p g d", d=HD),
                                func=AF.Identity, bias=0.0,
                                scale=rcol_p[:, st:st + 1])
                            nc.vector.scalar_tensor_tensor(
                                out=vsl,
                                in0=wvs_b[:, jc * SCW:(jc + 1) * SCW].rearrange(
                                    "p (g d) -> p g d", d=HD),
                                scalar=mrcol_p[:, st:st + 1], in1=vsl,
                                op0=OP.mult, op1=OP.add)
                        if deferred_mms is not None:
                            deferred_mms()
                            deferred_mms = None

                wv_next = None
                # Wo prefetch: first two tiles' DMAs issued before attention so
                # phase C's first matmul chains never wait on the transfer
                wo_pre = []
                for jt in range(2):
                    wo_sb = wop.tile([P, HT, P], bf16, tag="wo")
                    nc.sync.dma_start(out=wo_sb[:], in_=wo_t[l, jt])
                    wo_pre.append(wo_sb)

                def emit_qk(jt):
                    wq_sb = wp8.tile([P, HT, P], bf16, tag="w8")
                    nc.sync.dma_start(out=wq_sb[:], in_=wq_t[l, jt])
                    wk_sb = wp8.tile([P, HT, P], bf16, tag="w8")
                    nc.sync.dma_start(out=wk_sb[:], in_=wk_t[l, jt])
                    for sc in range(NSC):
                        ssl = slice(sc * SCW, (sc + 1) * SCW)
                        for (w_sb, dst, b_col, s_col) in (
                            (wq_sb, qT, bq_sb, wqs_sb),
                            (wk_sb, kT, bk_sb, wks_sb),
                        ):
                            pq = psp.tile([P, 512], f32, tag="mm", name=f"p_{jt}_{sc}")
                            for ko in range(HT):
                                rhs = (x_bf[:, ko, ssl] if prev_ln2 is None
                                       else prev_ln2[sc][0][:, ko, :])
                                nc.tensor.matmul(pq[:], w_sb[:, ko, :], rhs,
                                                 start=(ko == 0), stop=(ko == HT - 1))
                            if prev_ln2 is not None:
                                nc.vector.tensor_tensor(
                                    out=pq[:], in0=pq[:], in1=prev_ln2[sc][1][:],
                                    op=OP.mult)
                                nc.vector.scalar_tensor_tensor(
                                    out=pq[:], in0=prev_ln2[sc][2][:],
                                    scalar=s_col[:, jt:jt + 1], in1=pq[:],
                                    op0=OP.mult, op1=OP.add)
                            # bias-add + bf16 cast on ACT (idle in this phase)
                            # instead of DVE, which is near-saturated here
                            nc.scalar.activation(out=dst[:, jt, ssl], in_=pq[:],
                                                 func=AF.Identity,
                                                 bias=b_col[:, jt:jt + 1], scale=1.0)

                exp_tiles = {}

                def emit_scores(pair, qc):
                    jt = pair
                    qsl = slice(qc * SCW, (qc + 1) * SCW)
                    expA = expp.tile([P, HT, SCW], bf16, tag="expA")
                    expB = expp.tile([P, HT, SCW], bf16, tag="expB")
                    exp_tiles[(pair, qc)] = (expA, expB)
                    for kcg in range(4):
                        psA = pssc.tile([P, 1024], f32, tag="sc")
                        psB = pssc.tile([P, 1024], f32, tag="sc")
                        for ki in range(2):
                            kc = kcg * 2 + ki
                            ksl = slice(kc * P, (kc + 1) * P)
                            nc.tensor.matmul(psA[:, ki * 512:(ki + 1) * 512],
                                             kT[0:HD, jt, ksl], qT[0:HD, jt, qsl],
                                             start=True, stop=True)
                            nc.tensor.matmul(psB[:, ki * 512:(ki + 1) * 512],
                                             kT[HD:P, jt, ksl], qT[HD:P, jt, qsl],
                                             start=True, stop=True)
                        nc.scalar.activation(
                            out=expA[:, kcg * 2:(kcg + 1) * 2, :].rearrange("p a b -> p (a b)"),
                            in_=psA[:], func=AF.Exp, bias=0.0, scale=1.0 / np.sqrt(HD))
                        nc.scalar.activation(
                            out=expB[:, kcg * 2:(kcg + 1) * 2, :].rearrange("p a b -> p (a b)"),
                            in_=psB[:], func=AF.Exp, bias=0.0, scale=1.0 / np.sqrt(HD))

                def emit_attnv(pair, qc):
                    qsl = slice(qc * SCW, (qc + 1) * SCW)
                    expA, expB = exp_tiles.pop((pair, qc))
                    # [o | Z] at psum rows [0:64 | 64]; attnV psums use the
                    # (idle during attention) stat-psum slots so they don't
                    # contend with the Q/K projection copy-outs
                    poA = psst.tile([P, 512], f32, tag="st", name="poA")
                    for kt in range(HT):
                        nc.tensor.matmul(poA[0:HD + 1, :], v_aug[:, kt, 2 * pair, :],
                                         expA[:, kt, :],
                                         start=(kt == 0), stop=(kt == HT - 1))
                    rzA = rowp.tile([1, SCW], bf16, tag="rz")
                    with nc.allow_low_precision(reason="1/Z row cast to bf16 to match bf16 attn weights"):
                        nc.vector.reciprocal(rzA[:], poA[HD:HD + 1, :])
                    rzbA = rowp.tile([P, SCW], bf16, tag="rzb")
                    _bcast_row(nc, rzbA[:, :], rzA[:], P)
                    nc.vector.tensor_tensor(out=oT[0:HD, pair, qsl], in0=poA[0:HD, :],
                                            in1=rzbA[0:HD, :], op=OP.mult)
                    # odd head: same layout; result is DMA-shifted to rows 64:128
                    poB = psst.tile([P, 512], f32, tag="st", name="poB")
                    for kt in range(HT):
                        nc.tensor.matmul(poB[0:HD + 1, :], v_aug[:, kt, 2 * pair + 1, :],
                                         expB[:, kt, :],
                                         start=(kt == 0), stop=(kt == HT - 1))
                    rzB = rowp.tile([1, SCW], bf16, tag="rz")
                    with nc.allow_low_precision(reason="1/Z row cast to bf16 to match bf16 attn weights"):
                        nc.vector.reciprocal(rzB[:], poB[HD:HD + 1, :])
                    rzbB = rowp.tile([P, SCW], bf16, tag="rzb")
                    _bcast_row(nc, rzbB[:, :], rzB[:], P)
                    tmpB = rowp.tile([P, SCW], bf16, tag="tmpb")
                    nc.vector.tensor_tensor(out=tmpB[0:HD, :], in0=poB[0:HD, :],
                                            in1=rzbB[0:HD, :], op=OP.mult)
                    nc.sync.dma_start(out=oT[HD:P, pair, qsl], in_=tmpB[0:HD, :])

                # steady state per iter: attnV(p-3) | Q/K(p) | scores(p-1).
                # exp(p-1) runs on ACT during the following iter's Q/K+scores
                # window; attnV(p-1) only needs it two iters later.
                for p in range(NPAIR + 3):
                    # the two attnV chains are spaced apart in the iter so each
                    # [o|Z] psum drains (recip -> bcast -> mult) under the Q/K
                    # and scores matmuls instead of gating the slot reuse
                    if 3 <= p:
                        emit_attnv(p - 3, 0)
                        emit_attnv(p - 3, 1)
                    if p < NPAIR:
                        emit_qk(p)
                    if p == 2 and prev_ln2 is not None:
                        # LN2 apply to the fp32 residual: DVE/Pool work that
                        # rides along under the attention pipeline
                        emit_ln_apply(prev_ln2, pg2_sb, pc2_sb, None)
                    if 1 <= p <= NPAIR:
                        emit_scores(p - 1, 0)
                        emit_scores(p - 1, 1)

                # ---- phase C: Wo projection + residual + LN1
                # sc-outer; each chunk's LN stats/rows emitted right after its
                # residual so they run during the other chunk's matmuls
                if l < L - 1:
                    # prefetch BOTH of the next layer's V-weight chunks: the
                    # jc1 transfer was landing exactly in the layer-boundary
                    # stall when issued from inside the V loop
                    wv_next = []
                    for jc in range(NSC):
                        wv_nx = wpv.tile([P, HT, SCW], bf16, tag="wv",
                                         name=f"wv_next{jc}")
                        nc.sync.dma_start(
                            out=wv_nx[:],
                            in_=wv_t[l + 1, :, :, jc * SCW:(jc + 1) * SCW])
                        wv_next.append(wv_nx)
                rbp1 = rowp2.tile([P, NSC, SCW], bf16, tag="rb")
                mrbp1 = rowp2.tile([P, NSC, SCW], bf16, tag="mrb")
                ln1 = []
                for sc in range(NSC):
                    ssl = slice(sc * SCW, (sc + 1) * SCW)
                    # chunk 1's tb reuses the expA slot (attention is done with
                    # it by phase C) instead of a dedicated 8KB tag
                    st1 = ln_begin(sc, ("ktx2", "expA")[sc], "big")
                    for jt in range(HT):
                        if sc == 0 and jt < 2:
                            wo_sb = wo_pre[jt]
                        else:
                            wo_sb = wop.tile([P, HT, P], bf16, tag="wo")
                            nc.sync.dma_start(out=wo_sb[:], in_=wo_t[l, jt])
                        po = psp.tile([P, 512], f32, tag="mm")
                        for ko in range(HT):
                            nc.tensor.matmul(po[:], wo_sb[:, ko, :], oT[:, ko, ssl],
                                             start=(ko == 0), stop=(ko == HT - 1))
                        nc.vector.scalar_tensor_tensor(
                            out=xT[:, jt, ssl], in0=po[:], scalar=bo_sb[:, jt:jt + 1],
                            in1=xT[:, jt, ssl], op0=OP.add, op1=OP.add)
                        # stats for h-tile jt-1 fold in one tile behind the
                        # residual producer: the PE stat matmuls never wait
                        if jt >= 1:
                            ln_kt(st1, sc, jt - 1)
                    ln_kt(st1, sc, HT - 1)
                    tb = ln_end(st1, sc, rbp1, mrbp1)
                    ln1.append((tb, rbp1[:, sc, :], mrbp1[:, sc, :]))
                ln1.append((rbp1, mrbp1))

                # ---- phase D: FFN + residual + LN2, one s-chunk at a time.
                # Halving the fT buffer (W1 streamed twice) frees the SBUF for
                # LN2's dedicated tb pool, which the attention pipeline needs.
                # FFN1 runs on the pre-LN tensor; LN1's affine is folded into
                # the PSUM copy-out, so TensorE never waits for the LN1 chain.
                rbp2 = rowp2.tile([P, NSC, SCW], bf16, tag="rb")
                mrbp2 = rowp2.tile([P, NSC, SCW], bf16, tag="mrb")
                ln2 = []
                if l == L - 1:
                    out_sb = xbfop.tile([P, HT, S], bf16, tag="xbfo")
                    rcol_n = mrcol_n = None
                else:
                    # rstd / mean*rstd as per-partition fp32 columns for the
                    # next layer's deferred-LN V projection
                    rcol_n = csp.tile([P, HT], f32, tag="rcol")
                    mrcol_n = csp.tile([P, HT], f32, tag="mrcol")
                for sc in range(NSC):
                    ssl = slice(sc * SCW, (sc + 1) * SCW)
                    fT = bigp.tile([P, PT, SCW], bf16, tag="big", name="fT")
                    for pt in range(PT):
                        w1_sb = wp8.tile([P, HT, P], bf16, tag="w8")
                        nc.sync.dma_start(out=w1_sb[:], in_=w1_t[l, pt])
                        # four half-tile chains in flight (2 psp + 2 pssc
                        # slots): the relu/affine drain of one chain hides
                        # under the next chains' matmuls
                        if pt % 2 == 0:
                            pf = psp.tile([P, 512], f32, tag="mm",
                                          name=f"pf_{pt}")[:, :SCW]
                        else:
                            pf = pssc.tile([P, 1024], f32, tag="sc",
                                           name=f"pf_{pt}")[:, :SCW]
                        tb = ln1[sc][0]
                        for ko in range(HT):
                            nc.tensor.matmul(pf, w1_sb[:, ko, :], tb[:, ko, :],
                                             start=(ko == 0), stop=(ko == HT - 1))
                        nc.vector.tensor_tensor(out=pf, in0=pf,
                                                in1=ln1[sc][1][:], op=OP.mult)
                        nc.vector.scalar_tensor_tensor(
                            out=pf, in0=ln1[sc][2][:],
                            scalar=w1s_sb[:, pt:pt + 1], in1=pf,
                            op0=OP.mult, op1=OP.add)
                        nc.scalar.activation(
                            out=fT[:, pt, :], in_=pf,
                            func=AF.Relu, bias=b1_sb[:, pt:pt + 1], scale=1.0)
                    # LN1's fp32 xT update for this chunk rides under FFN2
                    emit_ln_apply(ln1, g1_sb, c1_sb, None, scs=[sc])
                    st2 = ln_begin(sc, "tb2", "ktx2")
                    for jt in range(HT):
                        pf2 = psp.tile([P, 512], f32, tag="mm")
                        for half in range(2):
                            # half-W2 tiles on the ACT DGE queue (idle during
                            # FFN2): halves the issue count and keeps the SP
                            # queue free for W1/stores
                            w2_sb = wp2.tile([P, PT // 2, P], bf16, tag="w16")
                            nc.scalar.dma_start(
                                out=w2_sb[:],
                                in_=w2_t[l, jt, :, half * 16:(half + 1) * 16, :])
                            for ki in range(PT // 2):
                                ko = half * 16 + ki
                                nc.tensor.matmul(pf2[:], w2_sb[:, ki, :], fT[:, ko, :],
                                                 start=(ko == 0), stop=(ko == PT - 1))
                        nc.vector.scalar_tensor_tensor(
                            out=xT[:, jt, ssl], in0=pf2[:], scalar=b2_sb[:, jt:jt + 1],
                            in1=xT[:, jt, ssl], op0=OP.add, op1=OP.add)
                        if jt >= 1:
                            ln_kt(st2, sc, jt - 1)
                    ln_kt(st2, sc, HT - 1)
                    tb = ln_end(st2, sc, rbp2, mrbp2, cols=(rcol_n, mrcol_n))
                    ln2.append((tb, rbp2[:, sc, :], mrbp2[:, sc, :]))
                    if l == L - 1:
                        emit_ln_apply(ln2, g2_sb, c2_sb, None, out_sb=out_sb,
                                      scs=[sc])
                ln2.append((rbp2, mrbp2))
                if l < L - 1:
                    prev_ln2 = ln2
                    pg2_sb, pc2_sb = g2_sb, c2_sb
                    rcol_p, mrcol_p = rcol_n, mrcol_n

    nc.finalize()
    return nc


_CACHE = {}


def _get_nc():
    if "nc" not in _CACHE:
        _CACHE["nc"] = build_nc()
    return _CACHE["nc"]


def _prep_inputs(inputs):
    bf = ml_dtypes.bfloat16
    src = np.asarray(inputs["src"]).astype(np.int32)  # [B, S]
    tok = np.ascontiguousarray(np.asarray(inputs["tok_emb"], dtype=np.float32))
    pos = np.asarray(inputs["pos_emb"], dtype=np.float32)  # [S, H]
    # posT[p, ht, s] = pos[s, ht*128+p]
    posT = np.ascontiguousarray(pos.T.reshape(HT, P, S).transpose(1, 0, 2)).astype(bf)

    def wsq(w):  # [L, H, H] -> [L, jt, p, ko, j]
        return np.ascontiguousarray(
            np.asarray(w, np.float32).reshape(L, HT, P, HT, P).transpose(0, 3, 2, 1, 4)
        ).astype(bf)

    g2_full = np.asarray(inputs["ln2_g"], np.float32)  # [L, H]
    c2_full = np.asarray(inputs["ln2_b"], np.float32)

    def defer_qk(w_raw, b_raw):
        """Fold the previous layer's ln2 affine into W (layers 1+)."""
        w = np.asarray(w_raw, np.float32).copy()  # [L, H, H]
        b_eff = np.asarray(b_raw, np.float32).copy()  # [L, H]
        for l in range(1, L):
            b_eff[l] = b_eff[l] + c2_full[l - 1] @ w[l]
            w[l] = w[l] * g2_full[l - 1][:, None]
        wsum_neg = -w.astype(bf).astype(np.float32).sum(axis=1)  # [L, H]
        wsum_neg[0] = 0.0
        return w, b_eff, wsum_neg

    wq_f, bq_eff, wqs = defer_qk(inputs["Wq"], inputs["bq"])
    wk_f, bk_eff, wks = defer_qk(inputs["Wk"], inputs["bk"])
    wq = wsq(wq_f)
    wk = wsq(wk_f)
    wo = wsq(inputs["Wo"])
    # V projection is deferred like Q/K: fold the previous layer's ln2 gamma
    # into Wv; the beta/bias constant rides through the softmax into bo.
    wv_eff = np.asarray(inputs["Wv"], np.float32).copy()  # [L, H, H]
    bv_full = np.asarray(inputs["bv"], np.float32)
    vconst = bv_full.copy()  # [L, H] constant-per-j part of v
    for l in range(1, L):
        vconst[l] = vconst[l] + c2_full[l - 1] @ wv_eff[l]
        wv_eff[l] = wv_eff[l] * g2_full[l - 1][:, None]
    wv_bf = wv_eff.astype(bf)
    wvs = np.zeros((L, H), np.float32)
    wvs[1:] = -wv_bf.astype(np.float32)[1:].sum(axis=1)
    # wv[l, p, ko, j] = Wv'[l, ko*128+p, j]
    wv = np.ascontiguousarray(
        wv_bf.reshape(L, HT, P, H).transpose(0, 2, 1, 3))
    g1_full = np.asarray(inputs["ln1_g"], np.float32)  # [L, H]
    c1_full = np.asarray(inputs["ln1_b"], np.float32)
    w1_full = np.asarray(inputs["W1"], np.float32)  # [L, H, PF]
    w1_eff = w1_full * g1_full[:, :, None]
    w1 = np.ascontiguousarray(
        w1_eff.reshape(L, HT, P, PT, P).transpose(0, 3, 2, 1, 4)
    ).astype(bf)
    w1_sum_neg = -w1_eff.astype(bf).astype(np.float32).sum(axis=1)  # [L, PF]
    b1_eff = np.asarray(inputs["b1"], np.float32) + np.einsum(
        "lh,lhp->lp", c1_full, w1_full)
    w2 = np.ascontiguousarray(
        np.asarray(inputs["W2"], np.float32).reshape(L, PT, P, HT, P).transpose(0, 3, 2, 1, 4)
    ).astype(bf)

    def colmajor(b, nt):  # [L, nt*128] -> [L, p, nt]
        return np.ascontiguousarray(
            np.asarray(b, np.float32).reshape(L, nt, P).transpose(0, 2, 1))

    wo_full = np.asarray(inputs["Wo"], np.float32)
    bo_eff = np.asarray(inputs["bo"], np.float32) + np.einsum(
        "lh,lhj->lj", vconst, wo_full)

    common = {
        "emb": tok,
        "post": posT,
        "wq": wq, "wk": wk, "wv": wv, "wo": wo, "w1": w1, "w2": w2,
        "wvs": wvs.astype(bf),
        "bq": colmajor(bq_eff, HT),
        "bk": colmajor(bk_eff, HT),
        "wqs": colmajor(wqs, HT),
        "wks": colmajor(wks, HT),
        "bo": colmajor(bo_eff, HT),
        "b1": colmajor(b1_eff, PT),
        "w1s": colmajor(w1_sum_neg, PT),
        "b2": colmajor(inputs["b2"], HT),
        "g1": colmajor(inputs["ln1_g"], HT),
        "c1": colmajor(inputs["ln1_b"], HT),
        "g2": colmajor(inputs["ln2_g"], HT),
        "c2": colmajor(inputs["ln2_b"], HT),
    }
    in_maps = []
    for b in range(B):
        m = dict(common)
        m["src"] = np.ascontiguousarray(src[b])
        in_maps.append(m)
    return in_maps


def _run(inputs, trace=False):
    nc = _get_nc()
    in_maps = _prep_inputs(inputs)
    res = run_bass_kernel_spmd(nc, in_maps, core_ids=list(range(B)), trace=trace)
    outs = []
    for r in res.results:
        ot = np.asarray(r["outT"]).astype(np.float32)  # [P, HT, S] bf16
        outs.append(ot.transpose(2, 1, 0).reshape(S, H))
    return np.stack(outs, axis=0), res


def kernel(**inputs):
    out, _ = _run(inputs, trace=False)
    return out


def kernel_traced(**inputs):
    return _run(inputs, trace=True)



# revision 84
# speedup vs baseline: 1.0085x; 1.0085x over previous
"""Trainium2 Bass kernel for a 6-layer post-LN transformer encoder.

Problem: B=8, S=1024, H=1024, NH=16 heads (HD=64), PF=4096, L=6, V=32000.
Sharding: pure data-parallel over batch -- each of the 8 NeuronCores runs one
batch element end-to-end (no collectives).

Device-side layout: activations are kept TRANSPOSED, xT[h, s], stored in SBUF
as [128 partitions, 8 h-tiles, 1024 s].  With weights in natural [h_in, h_out]
layout as the stationary matmul operand, every projection's output comes out
transposed as well, so the entire layer chains matmuls with zero on-device
transposes:
  qT/kT  = W.T @ xT            (lhsT = W[h,j],  rhs = xT)         -> [j, s]
  v      = xT.T @ Wv           (lhsT = xT tile, rhs = Wv[h,j])    -> [s, j]
  scoresT= kT_head.T @ qT_head (K = 64 head dims)                 -> [k, q]
  expT   = exp(scoresT / 8)    (no max-subtraction needed: |scores|<~3)
  oT,Z   = [v_head|1].T @ expT (ones column gives softmax denom Z) -> [d+1, q]
  attn   = Wo.T @ oT ;  FFN the same way.
LayerNorm is over the partition dim; stats come from ones-vector matmuls and
rstd from a DVE Newton rsqrt (ScalarE never leaves the exp table set).  The
LN affine is DEFERRED into the consumers' PSUM copy-outs (x2 @ W =
r*(t @ Wg) - (m*r)*colsum(Wg) with g/c folded into W/b on the host), so
TensorE runs the next phase's matmuls on the pre-LN tensor and never waits
for the LN chain.  Matmuls run in bf16 with fp32 PSUM accumulation; the
residual stream is fp32.
"""

import sys

import numpy as np

for _p in ("/opt/pypackages", "/opt/trn_rl_repo"):
    if _p not in sys.path:
        sys.path.insert(0, _p)

import ml_dtypes  # noqa: E402

import concourse.bass as bass  # noqa: E402
import concourse.bacc as bacc  # noqa: E402
import concourse.mybir as mybir  # noqa: E402
import concourse.tile as tile  # noqa: E402
from concourse.bass_utils import run_bass_kernel_spmd  # noqa: E402
from concourse.masks import make_identity  # noqa: E402

B, S, H, NH, PF, L, V = 8, 1024, 1024, 16, 4096, 6, 32000
HD = H // NH  # 64
P = 128
HT = H // P  # 8 h-tiles
PT = PF // P  # 32 pf-tiles
NSC = 2
SCW = S // NSC  # 512
NPAIR = NH // 2  # 8 head pairs
EPS = 1e-5

f32 = mybir.dt.float32
bf16 = mybir.dt.bfloat16
i32 = mybir.dt.int32
OP = mybir.AluOpType
AF = mybir.ActivationFunctionType


def _bcast_row(nc, out_ap, row_ap, rows):
    """Replicate a [1, N] SBUF row across `rows` partitions of out_ap."""
    nc.gpsimd.partition_broadcast(out_ap, row_ap)


def build_nc():
    nc = bacc.Bacc("TRN2")

    src_t = nc.dram_tensor("src", [S], i32, kind="ExternalInput")
    emb_t = nc.dram_tensor("emb", [V, H], f32, kind="ExternalInput")
    pos_t = nc.dram_tensor("post", [P, HT, S], bf16, kind="ExternalInput")
    wq_t = nc.dram_tensor("wq", [L, HT, P, HT, P], bf16, kind="ExternalInput")
    wk_t = nc.dram_tensor("wk", [L, HT, P, HT, P], bf16, kind="ExternalInput")
    wv_t = nc.dram_tensor("wv", [L, P, HT, H], bf16, kind="ExternalInput")
    # negated column sums of (g2-folded) Wv for the deferred-LN V projection
    wvs_t = nc.dram_tensor("wvs", [L, H], bf16, kind="ExternalInput")
    wo_t = nc.dram_tensor("wo", [L, HT, P, HT, P], bf16, kind="ExternalInput")
    w1_t = nc.dram_tensor("w1", [L, PT, P, HT, P], bf16, kind="ExternalInput")
    w2_t = nc.dram_tensor("w2", [L, HT, P, PT, P], bf16, kind="ExternalInput")
    bq_t = nc.dram_tensor("bq", [L, P, HT], f32, kind="ExternalInput")
    bk_t = nc.dram_tensor("bk", [L, P, HT], f32, kind="ExternalInput")
    bo_t = nc.dram_tensor("bo", [L, P, HT], f32, kind="ExternalInput")
    b1_t = nc.dram_tensor("b1", [L, P, PT], f32, kind="ExternalInput")
    w1s_t = nc.dram_tensor("w1s", [L, P, PT], f32, kind="ExternalInput")
    wqs_t = nc.dram_tensor("wqs", [L, P, HT], f32, kind="ExternalInput")
    wks_t = nc.dram_tensor("wks", [L, P, HT], f32, kind="ExternalInput")
    b2_t = nc.dram_tensor("b2", [L, P, HT], f32, kind="ExternalInput")
    g1_t = nc.dram_tensor("g1", [L, P, HT], f32, kind="ExternalInput")
    c1_t = nc.dram_tensor("c1", [L, P, HT], f32, kind="ExternalInput")
    g2_t = nc.dram_tensor("g2", [L, P, HT], f32, kind="ExternalInput")
    c2_t = nc.dram_tensor("c2", [L, P, HT], f32, kind="ExternalInput")
    # bf16 output (host upcasts): halves the store DMA and lets the final
    # tensor_scalar cast in the same pass
    out_t = nc.dram_tensor("outT", [P, HT, S], bf16, kind="ExternalOutput")

    from contextlib import ExitStack

    with tile.TileContext(nc) as tc:
        with ExitStack() as ctx:
            constp = ctx.enter_context(tc.tile_pool(name="const", bufs=1))
            residp = ctx.enter_context(tc.tile_pool(name="resid", bufs=1))
            bigp = ctx.enter_context(tc.tile_pool(name="big", bufs=1))
            ktp = ctx.enter_context(tc.tile_pool(name="ktx2", bufs=1))
            tb2p = ctx.enter_context(tc.tile_pool(name="tb2", bufs=1))
            csp = ctx.enter_context(tc.tile_pool(name="cs", bufs=2))
            vaugp = ctx.enter_context(tc.tile_pool(name="vaug", bufs=1))
            xbfop = ctx.enter_context(tc.tile_pool(name="xbfo", bufs=1))
            expp = ctx.enter_context(tc.tile_pool(name="expp", bufs=1))
            wp8 = ctx.enter_context(tc.tile_pool(name="wp8", bufs=4))
            wop = ctx.enter_context(tc.tile_pool(name="wop", bufs=3))
            wpv = ctx.enter_context(tc.tile_pool(name="wpv", bufs=2))
            wp2 = ctx.enter_context(tc.tile_pool(name="wp2", bufs=2))
            rowp = ctx.enter_context(tc.tile_pool(name="rowp", bufs=1))
            rowp2 = ctx.enter_context(tc.tile_pool(name="rowp2", bufs=1))
            biasp = ctx.enter_context(tc.tile_pool(name="biasp", bufs=1))
            biasp2 = ctx.enter_context(tc.tile_pool(name="biasp2", bufs=2))
            psp = ctx.enter_context(tc.tile_pool(name="ps", bufs=2, space="PSUM"))
            pssc = ctx.enter_context(tc.tile_pool(name="ps_sc", bufs=2, space="PSUM"))
            psst = ctx.enter_context(tc.tile_pool(name="ps_st", bufs=2, space="PSUM"))
            ident = constp.tile([P, P], f32)
            make_identity(nc, ident[:])
            ones_bf = constp.tile([P, 1], bf16)
            nc.vector.memset(ones_bf[:], 1.0)

            xT = residp.tile([P, HT, S], f32)  # residual stream, updated in place

            # dependency-free warm-up matmuls: run during the first embedding
            # gather's ~15us DMA wait so the PE clock-gate (HAM) is already at
            # full rate when the real transposes arrive
            warm = psp.tile([P, 512], f32, tag="mm", name="warm")
            for _ in range(40):
                nc.tensor.transpose(warm[:, :P], ident[:], ident[:])

            def _emb_tile(g, st, posT, x_bf):
                # 4 transposes batched per PSUM tile; evac on ACT (idle here)
                # so the DVE only does the cheap pos add -- keeps the
                # embedding from being DVE- or evac-bound
                ssl = slice(st * P, (st + 1) * P)
                for hg in range(2):
                    pst = psp.tile([P, 512], f32, tag="mm")
                    for hi in range(4):
                        ht = hg * 4 + hi
                        nc.tensor.transpose(pst[:, hi * P:(hi + 1) * P],
                                            g[:, ht * P:(ht + 1) * P], ident[:])
                    hsl = slice(hg * 4, (hg + 1) * 4)
                    if (st + hg) % 2 == 0:
                        nc.scalar.activation(
                            out=xT[:, hsl, ssl],
                            in_=pst[:].rearrange("p (a b) -> p a b", b=P),
                            func=AF.Copy, bias=0.0, scale=float(np.sqrt(H)))
                        nc.vector.tensor_tensor(
                            out=xT[:, hsl, ssl], in0=xT[:, hsl, ssl],
                            in1=posT[:, hsl, ssl], op=OP.add)
                    else:
                        # alternate the PSUM evac between ACT and DVE so
                        # neither engine gates the transpose pipeline
                        nc.vector.scalar_tensor_tensor(
                            out=xT[:, hsl, ssl],
                            in0=pst[:].rearrange("p (a b) -> p a b", b=P),
                            scalar=float(np.sqrt(H)), in1=posT[:, hsl, ssl],
                            op0=OP.mult, op1=OP.add)

            # ---------------- embedding ----------------
            # indirect gathers first (slow per-row descriptors); pos DMA after
            # so it doesn't delay the first transpose's input
            posT = ktp.tile([P, HT, S], bf16, tag="ktx2")
            x_bf = xbfop.tile([P, HT, S], bf16, tag="xbfo")
            g_tiles = []
            for st in range(HT):
                idx = biasp.tile([P, 1], i32, tag="idx", bufs=4)
                nc.sync.dma_start(out=idx[:], in_=src_t[st * P:(st + 1) * P, None])
                if st % 2 == 0:
                    g = bigp.tile([P, H], f32, tag="big")
                else:
                    g = vaugp.tile([P, H], f32, tag="vaug")
                nc.gpsimd.indirect_dma_start(
                    out=g[:],
                    out_offset=None,
                    in_=emb_t[:, :],
                    in_offset=bass.IndirectOffsetOnAxis(ap=idx[:, :1], axis=0),
                )
                g_tiles.append(g)
                if st == 1:
                    nc.sync.dma_start(out=posT[:], in_=pos_t[:, :, :])
                if st < 2:
                    continue
                _emb_tile(g_tiles[st - 2], st - 2, posT, x_bf)
            _emb_tile(g_tiles[HT - 2], HT - 2, posT, x_bf)
            _emb_tile(g_tiles[HT - 1], HT - 1, posT, x_bf)
            for ht in range(HT):
                nc.vector.tensor_copy(out=x_bf[:, ht, :], in_=xT[:, ht, :])

            def ln_begin(sc, tb_tag, sq_tag):
                """Allocate the chunk's tb/sq tiles and the two stat-psum
                accumulators so stats can be fed incrementally per h-tile."""
                if tb_tag == "tb2":
                    tb = tb2p.tile([P, HT, SCW], bf16, tag=f"tb2_{sc}")
                elif tb_tag in ("expA", "expB"):
                    tb = expp.tile([P, HT, SCW], bf16, tag=tb_tag)
                else:
                    tb = ktp.tile([P, HT, SCW], bf16, tag=tb_tag)
                if sq_tag == "ktx2":
                    sq = ktp.tile([P, HT, SCW], bf16, tag="ktx2", name="sq")
                else:
                    sq = bigp.tile([P, HT, SCW], bf16, tag="big", name="sq")
                psm = psst.tile([1, SCW], f32, tag="st")
                pss = psst.tile([1, SCW], f32, tag="st")
                return (tb, sq, psm, pss)

            def ln_kt(state, sc, kt, defer_mms=False):
                """Fold h-tile kt of this chunk into the LN stats: bf16 copy,
                square (2x DVE mode), and one accumulating matmul per stat.
                Emitted one h-tile behind the residual producer so the PE
                never waits on the DVE copies."""
                tb, sq, psm, pss = state
                ssl = slice(sc * SCW, (sc + 1) * SCW)
                nc.vector.tensor_copy(out=tb[:, kt, :], in_=xT[:, kt, ssl])
                nc.vector.tensor_tensor(
                    out=sq[:, kt, :], in0=tb[:, kt, :], in1=tb[:, kt, :],
                    op=OP.mult)
                if defer_mms:
                    return
                nc.tensor.matmul(psm[:], ones_bf[:], tb[:, kt, :],
                                 start=(kt == 0), stop=(kt == HT - 1))
                nc.tensor.matmul(pss[:], ones_bf[:], sq[:, kt, :],
                                 start=(kt == 0), stop=(kt == HT - 1))

            def ln_end(state, sc, rbpair, mrbpair, cols=None):
                """Stat rows -> rstd/mean*rstd -> partition broadcasts."""
                tb, sq, psm, pss = state
                if True:
                    mrow = rowp.tile([1, SCW], f32, tag="mrow")
                    nc.vector.tensor_scalar(out=mrow[:], in0=psm[:], scalar1=1.0 / H,
                                            scalar2=None, op0=OP.mult)
                    m2 = rowp.tile([1, SCW], f32, tag="lntmp")
                    nc.vector.tensor_tensor(out=m2[:], in0=mrow[:], in1=mrow[:], op=OP.mult)
                    vrow = rowp.tile([1, SCW], f32, tag="vrow")
                    nc.vector.scalar_tensor_tensor(
                        out=vrow[:], in0=pss[:], scalar=1.0 / H, in1=m2[:],
                        op0=OP.mult, op1=OP.subtract)
                    nc.vector.tensor_scalar(out=vrow[:], in0=vrow[:], scalar1=EPS,
                                            scalar2=None, op0=OP.add)
                    # rstd = rsqrt(vrow): bit-hack seed + 2 Newton iterations
                    yrow = rowp.tile([1, SCW], f32, tag="yrow")
                    nc.vector.tensor_scalar(
                        out=yrow[:].bitcast(i32), in0=vrow[:].bitcast(i32),
                        scalar1=1, scalar2=None, op0=OP.logical_shift_right)
                    nc.vector.tensor_scalar(
                        out=yrow[:].bitcast(i32), in0=yrow[:].bitcast(i32),
                        scalar1=-1, scalar2=0x5F3759DF, op0=OP.mult, op1=OP.add)
                    srow = rowp.tile([1, SCW], f32, tag="lntmp")
                    for _ in range(1):
                        nc.vector.tensor_tensor(out=srow[:], in0=yrow[:], in1=yrow[:],
                                                op=OP.mult)
                        nc.vector.tensor_tensor(out=srow[:], in0=srow[:], in1=vrow[:],
                                                op=OP.mult)
                        nc.vector.tensor_scalar(out=srow[:], in0=srow[:], scalar1=-0.5,
                                                scalar2=1.5, op0=OP.mult, op1=OP.add)
                        nc.vector.tensor_tensor(out=yrow[:], in0=yrow[:], in1=srow[:],
                                                op=OP.mult)
                    if cols is not None and cols[0] is not None:
                        # fp32 m*r row (ACT scale APs must be FP32) feeding the
                        # per-partition column copies for the next layer's
                        # deferred-LN V projection
                        mrf = rowp.tile([1, SCW], f32, tag="mrf")
                        nc.vector.tensor_tensor(out=mrf[:], in0=mrow[:],
                                                in1=yrow[:], op=OP.mult)
                        mrrow = rowp.tile([1, SCW], bf16, tag="mrow2")
                        nc.vector.tensor_copy(out=mrrow[:], in_=mrf[:])
                        rcol_n, mrcol_n = cols
                        for st4 in range(4):
                            st = sc * 4 + st4
                            wsl = slice(st4 * P, (st4 + 1) * P)
                            nc.sync.dma_start(out=rcol_n[:, st:st + 1],
                                              in_=yrow[0:1, wsl])
                            nc.sync.dma_start(out=mrcol_n[:, st:st + 1],
                                              in_=mrf[0:1, wsl])
                    else:
                        mrrow = rowp.tile([1, SCW], bf16, tag="mrow2")
                        nc.vector.tensor_tensor(out=mrrow[:], in0=mrow[:],
                                                in1=yrow[:], op=OP.mult)
                    ybrow = rowp.tile([1, SCW], bf16, tag="rz")
                    nc.vector.tensor_copy(out=ybrow[:], in_=yrow[:])
                    _bcast_row(nc, rbpair[:, sc, :], ybrow[:], P)
                    _bcast_row(nc, mrbpair[:, sc, :], mrrow[:], P)
                    return tb

            def emit_ln_apply(handles, g_sb, c_sb, out_bf, out_sb=None, scs=None):
                """In-place fp32 LN apply: x2 = (x*r - m*r)*g + c.

                With out_sb set (final layer), the affine result is written to
                the bf16 out_sb tile and DMA'd out instead of updating xT
                (dead after the last layer); scs restricts to given chunks so
                chunk 0 overlaps chunk 1's FFN2 matmuls."""
                final = out_sb is not None
                for sc in (range(NSC) if scs is None else scs):
                    ssl = slice(sc * SCW, (sc + 1) * SCW)
                    _tb, rb, mrb = handles[sc]
                    # per-tile ops; odd tiles go to the (mostly idle) GpSimd
                    # engine -- on the final layer chunk 0 goes there entirely
                    # so the apply hides under chunk 1's FFN2 matmuls
                    def _eng(kt):
                        if final:
                            pool = (sc == 0) or kt >= 5
                        else:
                            pool = kt % 2 == 1
                        return nc.gpsimd if pool else nc.vector
                    for kt in range(HT):
                        _eng(kt).tensor_tensor(
                            out=xT[:, kt, ssl], in0=xT[:, kt, ssl],
                            in1=rb[:], op=OP.mult)
                        _eng(kt).tensor_tensor(
                            out=xT[:, kt, ssl], in0=xT[:, kt, ssl],
                            in1=mrb[:], op=OP.subtract)
                    # critical path first: bf16 LN output for consuming matmuls
                    for kt in range(HT):
                        if out_bf is not None:
                            nc.vector.tensor_scalar(
                                out=out_bf[:, kt, ssl], in0=xT[:, kt, ssl],
                                scalar1=g_sb[:, kt:kt + 1], scalar2=c_sb[:, kt:kt + 1],
                                op0=OP.mult, op1=OP.add)
                    for kt in range(HT):
                        _eng(kt).tensor_scalar(
                            out=(out_sb if final else xT)[:, kt, ssl],
                            in0=xT[:, kt, ssl],
                            scalar1=g_sb[:, kt:kt + 1], scalar2=c_sb[:, kt:kt + 1],
                            op0=OP.mult, op1=OP.add)
                        if final and kt == 3:
                            # first half streams out while the second half's
                            # affine is still running on the other engine
                            nc.sync.dma_start(out=out_t[:, 0:4, ssl],
                                              in_=out_sb[:, 0:4, ssl])
                    if final:
                        nc.sync.dma_start(out=out_t[:, 4:HT, ssl],
                                          in_=out_sb[:, 4:HT, ssl])

            # ---------------- layers ----------------
            prev_ln2 = None
            pg2_sb = pc2_sb = None
            rcol_p = mrcol_p = None
            deferred_mms = None
            wv_next = None
            for l in range(L):
                bq_sb = biasp.tile([P, HT], f32, tag="bq")
                nc.sync.dma_start(out=bq_sb[:], in_=bq_t[l])
                bk_sb = biasp.tile([P, HT], f32, tag="bk")
                nc.sync.dma_start(out=bk_sb[:], in_=bk_t[l])
                bo_sb = biasp.tile([P, HT], f32, tag="bo")
                nc.sync.dma_start(out=bo_sb[:], in_=bo_t[l])
                b1_sb = biasp.tile([P, PT], f32, tag="b1")
                nc.sync.dma_start(out=b1_sb[:], in_=b1_t[l])
                w1s_sb = biasp.tile([P, PT], f32, tag="w1s")
                nc.sync.dma_start(out=w1s_sb[:], in_=w1s_t[l])
                wqs_sb = biasp.tile([P, HT], f32, tag="wqs")
                nc.sync.dma_start(out=wqs_sb[:], in_=wqs_t[l])
                wks_sb = biasp.tile([P, HT], f32, tag="wks")
                nc.sync.dma_start(out=wks_sb[:], in_=wks_t[l])
                b2_sb = biasp.tile([P, HT], f32, tag="b2")
                nc.sync.dma_start(out=b2_sb[:], in_=b2_t[l])
                g1_sb = biasp.tile([P, HT], f32, tag="g1")
                nc.sync.dma_start(out=g1_sb[:], in_=g1_t[l])
                c1_sb = biasp.tile([P, HT], f32, tag="c1")
                nc.sync.dma_start(out=c1_sb[:], in_=c1_t[l])
                g2_sb = biasp2.tile([P, HT], f32, tag="g2")
                nc.sync.dma_start(out=g2_sb[:], in_=g2_t[l])
                c2_sb = biasp2.tile([P, HT], f32, tag="c2")
                nc.sync.dma_start(out=c2_sb[:], in_=c2_t[l])

                # ---- phase A+B: V, then a software pipeline interleaving the
                # Q/K projections with the attention inner loop so the exp
                # work (ScalarE-bound) spreads across the whole phase instead
                # of gating the PE in a bunched scores->exp->attnV chain.
                #
                # For l>=1 everything runs on the pre-LN tensor from the
                # previous layer's LN2 with the affine deferred into the PSUM
                # copy-outs (Q/K: folded scale rows; V: r/mr columns + Wv
                # colsum row), so nothing waits for the LN apply chain.
                qT = bigp.tile([P, HT, S], bf16, tag="big")
                kT = ktp.tile([P, HT, S], bf16, tag="ktx2")
                oT = xbfop.tile([P, HT, S], bf16, tag="xbfo")

                v_aug = vaugp.tile([P, HT, NH, HD + 1], bf16, tag="vaug")
                nc.vector.memset(v_aug[:, :, :, HD:HD + 1], 1.0)
                if prev_ln2 is not None:
                    wvs_row = rowp.tile([1, H], bf16, tag="wvsrow")
                    nc.sync.dma_start(out=wvs_row[:], in_=wvs_t[l, None, :])
                    wvs_b = csp.tile([P, H], bf16, tag="csb")
                    _bcast_row(nc, wvs_b[:, :], wvs_row[:], P)
                for jc in range(NSC):
                    if wv_next is not None:
                        # prefetched during the previous layer's phase C
                        wv_sb = wv_next[jc]
                    else:
                        wv_sb = wpv.tile([P, HT, SCW], bf16, tag="wv")
                        # ACT DGE queue: not stuck behind the W1/W2 stream on SP
                        nc.scalar.dma_start(out=wv_sb[:], in_=wv_t[l, :, :, jc * SCW:(jc + 1) * SCW])
                    for st in range(HT):
                        pv = psp.tile([P, 512], f32, tag="mm")
                        sc_of = st // 4
                        for ko in range(HT):
                            lhs = (x_bf[:, ko, st * P:(st + 1) * P]
                                   if prev_ln2 is None else
                                   prev_ln2[sc_of][0][:, ko, (st % 4) * P:(st % 4 + 1) * P])
                            nc.tensor.matmul(pv[:], lhs, wv_sb[:, ko, :],
                                             start=(ko == 0), stop=(ko == HT - 1))
                        vsl = v_aug[:, st, jc * 8:(jc + 1) * 8, 0:HD]
                        if prev_ln2 is None:
                            nc.scalar.activation(
                                out=vsl, in_=pv[:].rearrange("p (g d) -> p g d", d=HD),
                                func=AF.Copy, bias=0.0, scale=1.0)
                        else:
                            # v = r[s]*(t @ Wv') - mr[s]*colsum(Wv'): scale on
                            # ACT (per-partition r column), rank-1 fixup on the
                            # idle GpSimd so the DVE stays clear for Q/K evacs
                            nc.scalar.activation(
                                out=vsl, in_=pv[:].rearrange("p (g d) -> p g d", d=HD),
                                func=AF.Identity, bias=0.0,
                                scale=rcol_p[:, st:st + 1])
                            nc.vector.scalar_tensor_tensor(
                                out=vsl,
                                in0=wvs_b[:, jc * SCW:(jc + 1) * SCW].rearrange(
                                    "p (g d) -> p g d", d=HD),
                                scalar=mrcol_p[:, st:st + 1], in1=vsl,
                                op0=OP.mult, op1=OP.add)
                        if deferred_mms is not None:
                            deferred_mms()
                            deferred_mms = None

                wv_next = None
                # Wo prefetch: first two tiles' DMAs issued before attention so
                # phase C's first matmul chains never wait on the transfer
                wo_pre = []
                for jt in range(2):
                    wo_sb = wop.tile([P, HT, P], bf16, tag="wo")
                    nc.sync.dma_start(out=wo_sb[:], in_=wo_t[l, jt])
                    wo_pre.append(wo_sb)

                def emit_qk(jt):
                    wq_sb = wp8.tile([P, HT, P], bf16, tag="w8")
                    nc.sync.dma_start(out=wq_sb[:], in_=wq_t[l, jt])
                    wk_sb = wp8.tile([P, HT, P], bf16, tag="w8")
                    nc.sync.dma_start(out=wk_sb[:], in_=wk_t[l, jt])
                    for sc in range(NSC):
                        ssl = slice(sc * SCW, (sc + 1) * SCW)
                        for (w_sb, dst, b_col, s_col) in (
                            (wq_sb, qT, bq_sb, wqs_sb),
                            (wk_sb, kT, bk_sb, wks_sb),
                        ):
                            pq = psp.tile([P, 512], f32, tag="mm", name=f"p_{jt}_{sc}")
                            for ko in range(HT):
                                rhs = (x_bf[:, ko, ssl] if prev_ln2 is None
                                       else prev_ln2[sc][0][:, ko, :])
                                nc.tensor.matmul(pq[:], w_sb[:, ko, :], rhs,
                                                 start=(ko == 0), stop=(ko == HT - 1))
                            if prev_ln2 is not None:
                                nc.vector.tensor_tensor(
                                    out=pq[:], in0=pq[:], in1=prev_ln2[sc][1][:],
                                    op=OP.mult)
                                nc.vector.scalar_tensor_tensor(
                                    out=pq[:], in0=prev_ln2[sc][2][:],
                                    scalar=s_col[:, jt:jt + 1], in1=pq[:],
                                    op0=OP.mult, op1=OP.add)
                            # bias-add + bf16 cast on ACT (idle in this phase)
                            # instead of DVE, which is near-saturated here
                            nc.scalar.activation(out=dst[:, jt, ssl], in_=pq[:],
                                                 func=AF.Identity,
                                                 bias=b_col[:, jt:jt + 1], scale=1.0)

                exp_tiles = {}

                def emit_scores(pair, qc):
                    jt = pair
                    qsl = slice(qc * SCW, (qc + 1) * SCW)
                    expA = expp.tile([P, HT, SCW], bf16, tag="expA")
                    expB = expp.tile([P, HT, SCW], bf16, tag="expB")
                    exp_tiles[(pair, qc)] = (expA, expB)
                    for kcg in range(4):
                        psA = pssc.tile([P, 1024], f32, tag="sc")
                        psB = pssc.tile([P, 1024], f32, tag="sc")
                        for ki in range(2):
                            kc = kcg * 2 + ki
                            ksl = slice(kc * P, (kc + 1) * P)
                            nc.tensor.matmul(psA[:, ki * 512:(ki + 1) * 512],
                                             kT[0:HD, jt, ksl], qT[0:HD, jt, qsl],
                                             start=True, stop=True)
                            nc.tensor.matmul(psB[:, ki * 512:(ki + 1) * 512],
                                             kT[HD:P, jt, ksl], qT[HD:P, jt, qsl],
                                             start=True, stop=True)
                        nc.scalar.activation(
                            out=expA[:, kcg * 2:(kcg + 1) * 2, :].rearrange("p a b -> p (a b)"),
                            in_=psA[:], func=AF.Exp, bias=0.0, scale=1.0 / np.sqrt(HD))
                        nc.scalar.activation(
                            out=expB[:, kcg * 2:(kcg + 1) * 2, :].rearrange("p a b -> p (a b)"),
                            in_=psB[:], func=AF.Exp, bias=0.0, scale=1.0 / np.sqrt(HD))

                def emit_attnv(pair, qc):
                    qsl = slice(qc * SCW, (qc + 1) * SCW)
                    expA, expB = exp_tiles.pop((pair, qc))
                    # [o | Z] at psum rows [0:64 | 64]; attnV psums use the
                    # (idle during attention) stat-psum slots so they don't
                    # contend with the Q/K projection copy-outs
                    poA = psst.tile([P, 512], f32, tag="st", name="poA")
                    for kt in range(HT):
                        nc.tensor.matmul(poA[0:HD + 1, :], v_aug[:, kt, 2 * pair, :],
                                         expA[:, kt, :],
                                         start=(kt == 0), stop=(kt == HT - 1))
                    rzA = rowp.tile([1, SCW], bf16, tag="rz")
                    with nc.allow_low_precision(reason="1/Z row cast to bf16 to match bf16 attn weights"):
                        nc.vector.reciprocal(rzA[:], poA[HD:HD + 1, :])
                    rzbA = rowp.tile([P, SCW], bf16, tag="rzb")
                    _bcast_row(nc, rzbA[:, :], rzA[:], P)
                    nc.vector.tensor_tensor(out=oT[0:HD, pair, qsl], in0=poA[0:HD, :],
                                            in1=rzbA[0:HD, :], op=OP.mult)
                    # odd head: same layout; result is DMA-shifted to rows 64:128
                    poB = psst.tile([P, 512], f32, tag="st", name="poB")
                    for kt in range(HT):
                        nc.tensor.matmul(poB[0:HD + 1, :], v_aug[:, kt, 2 * pair + 1, :],
                                         expB[:, kt, :],
                                         start=(kt == 0), stop=(kt == HT - 1))
                    rzB = rowp.tile([1, SCW], bf16, tag="rz")
                    with nc.allow_low_precision(reason="1/Z row cast to bf16 to match bf16 attn weights"):
                        nc.vector.reciprocal(rzB[:], poB[HD:HD + 1, :])
                    rzbB = rowp.tile([P, SCW], bf16, tag="rzb")
                    _bcast_row(nc, rzbB[:, :], rzB[:], P)
                    tmpB = rowp.tile([P, SCW], bf16, tag="tmpb")
                    nc.vector.tensor_tensor(out=tmpB[0:HD, :], in0=poB[0:HD, :],
                                            in1=rzbB[0:HD, :], op=OP.mult)
                    nc.sync.dma_start(out=oT[HD:P, pair, qsl], in_=tmpB[0:HD, :])

                # steady state per iter: attnV(p-3) | Q/K(p) | scores(p-1).
                # exp(p-1) runs on ACT during the following iter's Q/K+scores
                # window; attnV(p-1) only needs it two iters later.
                for p in range(NPAIR + 3):
                    # the two attnV chains are spaced apart in the iter so each
                    # [o|Z] psum drains (recip -> bcast -> mult) under the Q/K
                    # and scores matmuls instead of gating the slot reuse
                    if 3 <= p:
                        emit_attnv(p - 3, 0)
                        emit_attnv(p - 3, 1)
                    if p < NPAIR:
                        emit_qk(p)
                    if p == 2 and prev_ln2 is not None:
                        # LN2 apply to the fp32 residual: DVE/Pool work that
                        # rides along under the attention pipeline
                        emit_ln_apply(prev_ln2, pg2_sb, pc2_sb, None)
                    if 1 <= p <= NPAIR:
                        emit_scores(p - 1, 0)
                        emit_scores(p - 1, 1)

                # ---- phase C: Wo projection + residual + LN1
                # sc-outer; each chunk's LN stats/rows emitted right after its
                # residual so they run during the other chunk's matmuls
                if l < L - 1:
                    # prefetch BOTH of the next layer's V-weight chunks: the
                    # jc1 transfer was landing exactly in the layer-boundary
                    # stall when issued from inside the V loop
                    wv_next = []
                    for jc in range(NSC):
                        wv_nx = wpv.tile([P, HT, SCW], bf16, tag="wv",
                                         name=f"wv_next{jc}")
                        nc.sync.dma_start(
                            out=wv_nx[:],
                            in_=wv_t[l + 1, :, :, jc * SCW:(jc + 1) * SCW])
                        wv_next.append(wv_nx)
                rbp1 = rowp2.tile([P, NSC, SCW], bf16, tag="rb")
                mrbp1 = rowp2.tile([P, NSC, SCW], bf16, tag="mrb")
                ln1 = []
                for sc in range(NSC):
                    ssl = slice(sc * SCW, (sc + 1) * SCW)
                    # chunk 1's tb reuses the expA slot (attention is done with
                    # it by phase C) instead of a dedicated 8KB tag
                    st1 = ln_begin(sc, ("ktx2", "expA")[sc], "big")
                    for jt in range(HT):
                        if sc == 0 and jt < 2:
                            wo_sb = wo_pre[jt]
                        else:
                            wo_sb = wop.tile([P, HT, P], bf16, tag="wo")
                            nc.sync.dma_start(out=wo_sb[:], in_=wo_t[l, jt])
                        po = psp.tile([P, 512], f32, tag="mm")
                        for ko in range(HT):
                            nc.tensor.matmul(po[:], wo_sb[:, ko, :], oT[:, ko, ssl],
                                             start=(ko == 0), stop=(ko == HT - 1))
                        nc.vector.scalar_tensor_tensor(
                            out=xT[:, jt, ssl], in0=po[:], scalar=bo_sb[:, jt:jt + 1],
                            in1=xT[:, jt, ssl], op0=OP.add, op1=OP.add)
                        # stats for h-tile jt-1 fold in one tile behind the
                        # residual producer: the PE stat matmuls never wait
                        if jt >= 1:
                            ln_kt(st1, sc, jt - 1)
                    ln_kt(st1, sc, HT - 1)
                    tb = ln_end(st1, sc, rbp1, mrbp1)
                    ln1.append((tb, rbp1[:, sc, :], mrbp1[:, sc, :]))
                ln1.append((rbp1, mrbp1))

                # ---- phase D: FFN + residual + LN2, one s-chunk at a time.
                # Halving the fT buffer (W1 streamed twice) frees the SBUF for
                # LN2's dedicated tb pool, which the attention pipeline needs.
                # FFN1 runs on the pre-LN tensor; LN1's affine is folded into
                # the PSUM copy-out, so TensorE never waits for the LN1 chain.
                rbp2 = rowp2.tile([P, NSC, SCW], bf16, tag="rb")
                mrbp2 = rowp2.tile([P, NSC, SCW], bf16, tag="mrb")
                ln2 = []
                if l == L - 1:
                    out_sb = xbfop.tile([P, HT, S], bf16, tag="xbfo")
                    rcol_n = mrcol_n = None
                else:
                    # rstd / mean*rstd as per-partition fp32 columns for the
                    # next layer's deferred-LN V projection
                    rcol_n = csp.tile([P, HT], f32, tag="rcol")
                    mrcol_n = csp.tile([P, HT], f32, tag="mrcol")
                for sc in range(NSC):
                    ssl = slice(sc * SCW, (sc + 1) * SCW)
                    fT = bigp.tile([P, PT, SCW], bf16, tag="big", name="fT")
                    for pt in range(PT):
                        w1_sb = wp8.tile([P, HT, P], bf16, tag="w8")
                        nc.sync.dma_start(out=w1_sb[:], in_=w1_t[l, pt])
                        # four half-tile chains in flight (2 psp + 2 pssc
                        # slots): the relu/affine drain of one chain hides
                        # under the next chains' matmuls
                        if pt % 2 == 0:
                            pf = psp.tile([P, 512], f32, tag="mm",
                                          name=f"pf_{pt}")[:, :SCW]
                        else:
                            pf = pssc.tile([P, 1024], f32, tag="sc",
                                           name=f"pf_{pt}")[:, :SCW]
                        tb = ln1[sc][0]
                        for ko in range(HT):
                            nc.tensor.matmul(pf, w1_sb[:, ko, :], tb[:, ko, :],
                                             start=(ko == 0), stop=(ko == HT - 1))
                        nc.vector.tensor_tensor(out=pf, in0=pf,
                                                in1=ln1[sc][1][:], op=OP.mult)
                        nc.vector.scalar_tensor_tensor(
                            out=pf, in0=ln1[sc][2][:],
                            scalar=w1s_sb[:, pt:pt + 1], in1=pf,
                            op0=OP.mult, op1=OP.add)
                        nc.scalar.activation(
                            out=fT[:, pt, :], in_=pf,
                            func=AF.Relu, bias=b1_sb[:, pt:pt + 1], scale=1.0)
                    # LN1's fp32 xT update for this chunk rides under FFN2
                    emit_ln_apply(ln1, g1_sb, c1_sb, None, scs=[sc])
                    st2 = ln_begin(sc, "tb2", "ktx2")
                    for jt in range(HT):
                        pf2 = psp.tile([P, 512], f32, tag="mm")
                        for half in range(2):
                            # half-W2 tiles on the ACT DGE queue (idle during
                            # FFN2): halves the issue count and keeps the SP
                            # queue free for W1/stores
                            w2_sb = wp2.tile([P, PT // 2, P], bf16, tag="w16")
                            nc.scalar.dma_start(
                                out=w2_sb[:],
                                in_=w2_t[l, jt, :, half * 16:(half + 1) * 16, :])
                            for ki in range(PT // 2):
                                ko = half * 16 + ki
                                nc.tensor.matmul(pf2[:], w2_sb[:, ki, :], fT[:, ko, :],
                                                 start=(ko == 0), stop=(ko == PT - 1))
                        nc.vector.scalar_tensor_tensor(
                            out=xT[:, jt, ssl], in0=pf2[:], scalar=b2_sb[:, jt:jt + 1],
                            in1=xT[:, jt, ssl], op0=OP.add, op1=OP.add)
                        if jt >= 1:
                            ln_kt(st2, sc, jt - 1)
                    ln_kt(st2, sc, HT - 1)
                    tb = ln_end(st2, sc, rbp2, mrbp2, cols=(rcol_n, mrcol_n))
                    ln2.append((tb, rbp2[:, sc, :], mrbp2[:, sc, :]))
                    if l == L - 1:
                        emit_ln_apply(ln2, g2_sb, c2_sb, None, out_sb=out_sb,
                                      scs=[sc])
                ln2.append((rbp2, mrbp2))
                if l < L - 1:
                    prev_ln2 = ln2
                    pg2_sb, pc2_sb = g2_sb, c2_sb
                    rcol_p, mrcol_p = rcol_n, mrcol_n

    nc.finalize()
    return nc


_CACHE = {}


def _get_nc():
    if "nc" not in _CACHE:
        _CACHE["nc"] = build_nc()
    return _CACHE["nc"]


def _prep_inputs(inputs):
    bf = ml_dtypes.bfloat16
    src = np.asarray(inputs["src"]).astype(np.int32)  # [B, S]
    tok = np.ascontiguousarray(np.asarray(inputs["tok_emb"], dtype=np.float32))
    pos = np.asarray(inputs["pos_emb"], dtype=np.float32)  # [S, H]
    # posT[p, ht, s] = pos[s, ht*128+p]
    posT = np.ascontiguousarray(pos.T.reshape(HT, P, S).transpose(1, 0, 2)).astype(bf)

    def wsq(w):  # [L, H, H] -> [L, jt, p, ko, j]
        return np.ascontiguousarray(
            np.asarray(w, np.float32).reshape(L, HT, P, HT, P).transpose(0, 3, 2, 1, 4)
        ).astype(bf)

    g2_full = np.asarray(inputs["ln2_g"], np.float32)  # [L, H]
    c2_full = np.asarray(inputs["ln2_b"], np.float32)

    def defer_qk(w_raw, b_raw):
        """Fold the previous layer's ln2 affine into W (layers 1+)."""
        w = np.asarray(w_raw, np.float32).copy()  # [L, H, H]
        b_eff = np.asarray(b_raw, np.float32).copy()  # [L, H]
        for l in range(1, L):
            b_eff[l] = b_eff[l] + c2_full[l - 1] @ w[l]
            w[l] = w[l] * g2_full[l - 1][:, None]
        wsum_neg = -w.astype(bf).astype(np.float32).sum(axis=1)  # [L, H]
        wsum_neg[0] = 0.0
        return w, b_eff, wsum_neg

    wq_f, bq_eff, wqs = defer_qk(inputs["Wq"], inputs["bq"])
    wk_f, bk_eff, wks = defer_qk(inputs["Wk"], inputs["bk"])
    wq = wsq(wq_f)
    wk = wsq(wk_f)
    wo = wsq(inputs["Wo"])
    # V projection is deferred like Q/K: fold the previous layer's ln2 gamma
    # into Wv; the beta/bias constant rides through the softmax into bo.
    wv_eff = np.asarray(inputs["Wv"], np.float32).copy()  # [L, H, H]
    bv_full = np.asarray(inputs["bv"], np.float32)
    vconst = bv_full.copy()  # [L, H] constant-per-j part of v
    for l in range(1, L):
        vconst[l] = vconst[l] + c2_full[l - 1] @ wv_eff[l]
        wv_eff[l] = wv_eff[l] * g2_full[l - 1][:, None]
    wv_bf = wv_eff.astype(bf)
    wvs = np.zeros((L, H), np.float32)
    wvs[1:] = -wv_bf.astype(np.float32)[1:].sum(axis=1)
    # wv[l, p, ko, j] = Wv'[l, ko*128+p, j]
    wv = np.ascontiguousarray(
        wv_bf.reshape(L, HT, P, H).transpose(0, 2, 1, 3))
    g1_full = np.asarray(inputs["ln1_g"], np.float32)  # [L, H]
    c1_full = np.asarray(inputs["ln1_b"], np.float32)
    w1_full = np.asarray(inputs["W1"], np.float32)  # [L, H, PF]
    w1_eff = w1_full * g1_full[:, :, None]
    w1 = np.ascontiguousarray(
        w1_eff.reshape(L, HT, P, PT, P).transpose(0, 3, 2, 1, 4)
    ).astype(bf)
    w1_sum_neg = -w1_eff.astype(bf).astype(np.float32).sum(axis=1)  # [L, PF]
    b1_eff = np.asarray(inputs["b1"], np.float32) + np.einsum(
        "lh,lhp->lp", c1_full, w1_full)
    w2 = np.ascontiguousarray(
        np.asarray(inputs["W2"], np.float32).reshape(L, PT, P, HT, P).transpose(0, 3, 2, 1, 4)
    ).astype(bf)

    def colmajor(b, nt):  # [L, nt*128] -> [L, p, nt]
        return np.ascontiguousarray(
            np.asarray(b, np.float32).reshape(L, nt, P).transpose(0, 2, 1))

    wo_full = np.asarray(inputs["Wo"], np.float32)
    bo_eff = np.asarray(inputs["bo"], np.float32) + np.einsum(
        "lh,lhj->lj", vconst, wo_full)

    common = {
        "emb": tok,
        "post": posT,
        "wq": wq, "wk": wk, "wv": wv, "wo": wo, "w1": w1, "w2": w2,
        "wvs": wvs.astype(bf),
        "bq": colmajor(bq_eff, HT),
        "bk": colmajor(bk_eff, HT),
        "wqs": colmajor(wqs, HT),
        "wks": colmajor(wks, HT),
        "bo": colmajor(bo_eff, HT),
        "b1": colmajor(b1_eff, PT),
        "w1s": colmajor(w1_sum_neg, PT),
        "b2": colmajor(inputs["b2"], HT),
        "g1": colmajor(inputs["ln1_g"], HT),
        "c1": colmajor(inputs["ln1_b"], HT),
        "g2": colmajor(inputs["ln2_g"], HT),
        "c2": colmajor(inputs["ln2_b"], HT),
    }
    in_maps = []
    for b in range(B):
        m = dict(common)
        m["src"] = np.ascontiguousarray(src[b])
        in_maps.append(m)
    return in_maps


def _run(inputs, trace=False):
    nc = _get_nc()
    in_maps = _prep_inputs(inputs)
    res = run_bass_kernel_spmd(nc, in_maps, core_ids=list(range(B)), trace=trace)
    outs = []
    for r in res.results:
        ot = np.asarray(r["outT"]).astype(np.float32)  # [P, HT, S] bf16
        outs.append(ot.transpose(2, 1, 0).reshape(S, H))
    return np.stack(outs, axis=0), res


def kernel(**inputs):
    out, _ = _run(inputs, trace=False)
    return out


def kernel_traced(**inputs):
    return _run(inputs, trace=True)

